# revision 17
# baseline (speedup 1.0000x reference)
"""Trainium2 Bass kernel for nn_AGILE_7696581394505 (gnn_message_passing).

5-layer GIN-style message passing over N=100k nodes / E=200k edges,
BN per layer, mean-pool over B=4096 graphs, MLP head.
Distributed over 8 NeuronCores: nodes sharded contiguously at graph
boundaries; per-layer h AllGather; BN stats AllReduce.

Device algorithm (all matmuls on TensorE, D-on-partition "transposed" MLP):
  aggr^T = table15_l^T-slices @ counts^T  (edge-embedding sums, K=16 matmul)
         + sum_chunks  M_chunk^T @ S_chunk  (gathered h[src] rows x 0/1 selector)
         + h^T (self-loop)
  y1^T = relu(W1^T-slices @ aggr^T + b1)   y2^T = W2^T @ y1^T + b2
  BN via free-axis sums + tiny AllReduce (+ pad-row correction), normalize as
  one scalar.activation (scale=a, bias=c, Relu fused).
  h^T -> PE-transpose -> node-major -> DRAM -> AllGather (next layer's gather).

kernel(**inputs) -> (hf [4096,512] f32, pred [4096,2] f32)
"""
import numpy as np
import ml_dtypes

import concourse.bass as bass
import concourse.mybir as mybir
from concourse import bacc
import concourse.tile as tile
from concourse.bass_utils import run_bass_kernel_spmd
from concourse.masks import make_identity
from concourse.tile import add_dep_helper

F32 = mybir.dt.float32
BF16 = mybir.dt.bfloat16
I32 = mybir.dt.int32
AF = mybir.ActivationFunctionType
ALU = mybir.AluOpType
BF = ml_dtypes.bfloat16

EPS = 1e-5
NCORES = 8


# --------------------------------------------------------------------------
# host preprocessing
# --------------------------------------------------------------------------

def make_plan(x, edge_index, edge_attr, batch, B, ncores=NCORES):
    N = x.shape[0]
    gcnt = np.bincount(batch, minlength=B)
    gcum = np.concatenate([[0], np.cumsum(gcnt)])
    gbound = [0]
    for d in range(1, ncores):
        target = N * d / ncores
        g = int(np.searchsorted(gcum, target))
        if g > 0 and abs(gcum[g - 1] - target) < abs(gcum[g] - target):
            g -= 1
        g = max(gbound[-1], min(g, B))
        gbound.append(g)
    gbound.append(B)
    nbound = [int(gcum[g]) for g in gbound]
    nreal = [nbound[d + 1] - nbound[d] for d in range(ncores)]
    greal = [gbound[d + 1] - gbound[d] for d in range(ncores)]

    m = max(nreal)
    pshard = ((m + 255) // 512 + 1) * 512          # mult of 512, >=128 pad slack
    NT = pshard // 128
    NST = NT // 4
    gtpad = ((max(greal) + 127) // 128 + 1) * 128
    GT = gtpad // 128

    owner = np.zeros(N, np.int32)
    gslot = np.zeros(N, np.int64)
    tile_of = np.zeros(N, np.int64)
    for d in range(ncores):
        sl = slice(nbound[d], nbound[d + 1])
        owner[sl] = d
        gslot[sl] = np.arange(nreal[d]) + d * pshard
        tile_of[sl] = np.arange(nreal[d]) // 128

    combo_e = (edge_attr[:, 0] * 3 + edge_attr[:, 1]).astype(np.int64)
    combo_x = (x[:, 0] * 3 + x[:, 1]).astype(np.int64)
    src, dst = edge_index[0].astype(np.int64), edge_index[1].astype(np.int64)
    edst_dev = owner[dst]

    NW = ncores // 2                      # gather source windows (2 shards each)
    win_of = owner // 2                   # window of a node's owner
    cnt_wtd = np.zeros((ncores, NW, NT), np.int64)
    np.add.at(cnt_wtd, (edst_dev, win_of[src], tile_of[dst]), 1)
    K_wt = np.ceil(cnt_wtd / 128).max(axis=0).astype(np.int64)   # [NW, NT]

    chunks = []     # (st, w, t_local) per chunk; per st: w-major, then tile
    st_off = []
    st_wspan = []   # [NST][NW] chunk count
    for st in range(NST):
        st_off.append(len(chunks))
        spans = []
        for w in range(NW):
            n0c = len(chunks)
            for tl in range(4):
                for _ in range(int(K_wt[w, st * 4 + tl])):
                    chunks.append((st, w, tl))
            spans.append(len(chunks) - n0c)
        st_wspan.append(spans)
    TOTCH = len(chunks)
    st_off.append(TOTCH)
    K4 = [st_off[i + 1] - st_off[i] for i in range(NST)]

    devs = []
    for d in range(ncores):
        n0, n1 = nbound[d], nbound[d + 1]
        nr = n1 - n0
        mask = edst_dev == d
        es, ed, ec = src[mask], dst[mask], combo_e[mask]
        ew = win_of[es]
        order = np.lexsort((ed - n0, ew))    # by (window, dst)
        es, ed, ec, ew = es[order], ed[order], ec[order], ew[order]
        ld = ed - n0
        lt = ld // 128

        # slot indices per (window, tile): gidx16 holds window-local rows
        gidx = np.zeros((128, TOTCH), np.int32)   # window-local src row
        selT = np.zeros((TOTCH, 128, 128), np.float32)
        cntT = np.zeros((16, pshard), np.float32)
        np.add.at(cntT, (ec, ld), 1.0)
        cntT[12, :nr] += 1.0
        oneT = np.zeros((16, pshard), np.float32)
        oneT[combo_x[n0:n1], np.arange(nr)] = 1.0

        key = ew * NT + lt
        kstart = np.searchsorted(key, np.arange(NW * NT + 1))
        ch_of_wt = {}
        for i, (st, w, tl) in enumerate(chunks):
            ch_of_wt.setdefault((w, st * 4 + tl), []).append(i)
        for w in range(NW):
            for t in range(NT):
                e0 = int(kstart[w * NT + t])
                e1 = int(kstart[w * NT + t + 1])
                if e1 == e0:
                    continue
                for k, ch in enumerate(ch_of_wt.get((w, t), [])):
                    a = e0 + k * 128
                    b = min(e0 + (k + 1) * 128, e1)
                    if a >= b:
                        break
                    lanes = np.arange(b - a)
                    gidx[lanes, ch] = (gslot[es[a:b]] - w * 2 * pshard).astype(
                        np.int32)
                    selT[ch, lanes, ld[a:b] - t * 128] = 1.0
        devs.append(dict(n0=n0, n1=n1, g0=gbound[d], g1=gbound[d + 1],
                         nr=nr, gr=greal[d], gidx=gidx, selT=selT,
                         cntT=cntT, oneT=oneT))

    # pooling bounds (compile-time shared)
    lo = [NT] * GT
    hi = [0] * GT
    for d in range(ncores):
        dv = devs[d]
        for gt in range(GT):
            ga = dv["g0"] + gt * 128
            gb = min(dv["g0"] + (gt + 1) * 128, dv["g1"])
            if ga >= dv["g1"]:
                continue
            na = int(gcum[ga]) - dv["n0"]
            nb = int(gcum[gb]) - dv["n0"]
            lo[gt] = min(lo[gt], na // 128)
            hi[gt] = max(hi[gt], (nb + 127) // 128)
    for gt in range(GT):
        if lo[gt] > hi[gt]:
            lo[gt], hi[gt] = 0, 0
    W = [hi[g] - lo[g] for g in range(GT)]
    SW = sum(W)

    for d in range(ncores):
        dv = devs[d]
        psel = np.zeros((max(SW, 1), 128, 128), np.float32)
        off = 0
        for gt in range(GT):
            for w in range(W[gt]):
                ncki = lo[gt] + w
                nloc = np.arange(ncki * 128, (ncki + 1) * 128)
                valid = nloc < dv["nr"]
                gl = np.full(128, -1, np.int64)
                nglob = dv["n0"] + nloc[valid]
                gl[valid] = batch[nglob] - dv["g0"] - gt * 128
                sel = (gl >= 0) & (gl < 128)
                lanes = np.arange(128)[sel]
                gcol = gl[sel]
                gglob = batch[dv["n0"] + nloc[sel]]
                psel[off + w, lanes, gcol] = 1.0 / np.maximum(gcnt[gglob], 1)
            off += W[gt]
        dv["poolsel"] = psel

    S = dict(N=N, B=B, ncores=ncores, pshard=pshard, NT=NT, NST=NST,
             gtpad=gtpad, GT=GT, TOTCH=TOTCH, K4=K4, st_off=st_off,
             chunks=chunks, NW=NW, st_wspan=st_wspan,
             pool_lo=lo, pool_W=W, SW=max(SW, 1))
    return S, devs


def make_tables(p, L, D):
    table0 = np.zeros((16, D), np.float32)
    for c in range(9):
        table0[c] = p["x_emb1"][c // 3] + p["x_emb2"][c % 3]
    table15 = np.zeros((L, 16, D), np.float32)
    for l in range(L):
        for c in range(15):
            table15[l, c] = p["edge_emb1"][l, c // 3] + p["edge_emb2"][l, c % 3]
    return table0, table15


def _lhsT_pack(Wm, KP, MP):
    """[K, M] weight -> [128, KP*MP] padded lhsT layout.
    col kc*MP + m holds Wm[kc*128 + kk, m] at partition kk."""
    K, M = Wm.shape
    nk = KP // 128 if KP % 128 == 0 else None
    nk = (KP + 127) // 128
    out = np.zeros((128, nk * MP), np.float32)
    for kc in range(nk):
        kk = min(128, K - kc * 128)
        if kk <= 0:
            break
        out[:kk, kc * MP:kc * MP + M] = Wm[kc * 128:kc * 128 + kk, :]
    return out


def _bias_pack(b, ngrp):
    out = np.zeros((128, ngrp), np.float32)
    for g in range(ngrp):
        n = min(128, len(b) - g * 128)
        if n <= 0:
            break
        out[:n, g] = b[g * 128:g * 128 + n]
    return out


def layout_params(p, L, D, FEAT):
    """Pack params into device layouts (f32; cast where needed later)."""
    D2 = 2 * D
    FH = FEAT // 2
    table0, table15 = make_tables(p, L, D)
    KD = ((D + 127) // 128) * 128       # 384
    KD2 = ((D2 + 127) // 128) * 128     # 640
    out = {}
    out["t0h"] = np.zeros((16, KD), np.float32)
    out["t0h"][:, :D] = table0
    out["t15h"] = np.zeros((L, 16, KD), np.float32)
    out["t15h"][:, :, :D] = table15
    out["w1h"] = np.stack([_lhsT_pack(np.asarray(p["W1"][l]), KD, KD2)
                           for l in range(L)])       # [L,128,3*640]
    out["w2h"] = np.stack([_lhsT_pack(np.asarray(p["W2"][l]), KD2, KD)
                           for l in range(L)])       # [L,128,5*384]
    out["b1h"] = np.stack([_bias_pack(np.asarray(p["b1"][l]), KD2 // 128)
                           for l in range(L)])
    out["b2h"] = np.stack([_bias_pack(np.asarray(p["b2"][l]), KD // 128)
                           for l in range(L)])
    out["gmh"] = np.stack([_bias_pack(np.asarray(p["gamma"][l]), KD // 128)
                           for l in range(L)])
    out["bth"] = np.stack([_bias_pack(np.asarray(p["beta"][l]), KD // 128)
                           for l in range(L)])
    out["wfh"] = _lhsT_pack(np.asarray(p["W_feat"]), KD, FEAT)   # [128, 3*512]
    out["bfh"] = _bias_pack(np.asarray(p["b_feat"]), FEAT // 128)
    out["wp1h"] = _lhsT_pack(np.asarray(p["Wp1"]), FEAT, FH)     # [128, 4*256]
    out["bp1h"] = _bias_pack(np.asarray(p["bp1"]), FH // 128)
    out["wp2h"] = _lhsT_pack(np.asarray(p["Wp2"]), FH, FH)       # [128, 2*256]
    out["bp2h"] = _bias_pack(np.asarray(p["bp2"]), FH // 128)
    out["wp3h"] = _lhsT_pack(np.asarray(p["Wp3"]), FH, 2)        # [128, 2*2]
    out["bp3h"] = np.asarray(p["bp3"], np.float32).reshape(2, 1)
    return out


# --------------------------------------------------------------------------
# device program
# --------------------------------------------------------------------------

def build_gnn(nc, S, L=5, D=300, FEAT=512, run_layers=None, debug_hb=False,
              debug_y2=False, debug_aggr=False):
    if run_layers is None:
        run_layers = L
    P = S["pshard"]
    NST, NT, GT = S["NST"], S["NT"], S["GT"]
    GTPAD = S["gtpad"]
    TOTCH, K4, st_off = S["TOTCH"], S["K4"], S["st_off"]
    chunks = S["chunks"]
    plo, pW, SW = S["pool_lo"], S["pool_W"], S["SW"]
    Ntot = S["N"]
    NC = S["ncores"]
    D2 = 2 * D
    FH = FEAT // 2
    DP = ((D + 191) // 128) * 128  # h row padded to 256B mult (384 for D=300)
    NW = S["NW"]
    st_wspan = S["st_wspan"]
    NG = (D + 127) // 128          # 3 channel groups
    NM = (D2 + 127) // 128         # 5 groups for y1
    NF = FEAT // 128               # 4
    NH = FH // 128                 # 2
    KD, KD2 = NG * 128, NM * 128
    dgw = [min(128, D - g * 128) for g in range(NG)]   # [128,128,44]

    # ---- dram I/O ----
    din = {}
    def dram_in(name, shape, dt):
        din[name] = nc.dram_tensor(name, shape, dt, kind="ExternalInput")
        return din[name]

    gidx_d = dram_in("gidx16", [128, TOTCH * 8], mybir.dt.int16)
    sel_d = dram_in("selTh", [128, TOTCH * 128], BF16)
    cnt_d = dram_in("cntT", [16, P], BF16)
    one_d = dram_in("oneT", [16, P], BF16)
    pool_d = dram_in("poolh", [128, SW * 128], BF16)
    t0_d = dram_in("t0h", [16, KD], BF16)
    t15_d = dram_in("t15h", [L, 16, KD], BF16)
    w1_d = dram_in("w1h", [L, 128, NG * KD2], BF16)
    w2_d = dram_in("w2h", [L, 128, NM * KD], BF16)
    b1_d = dram_in("b1h", [L, 128, NM], F32)
    b2_d = dram_in("b2h", [L, 128, NG], F32)
    gm_d = dram_in("gmh", [L, 128, NG], F32)
    bt_d = dram_in("bth", [L, 128, NG], F32)
    wf_d = dram_in("wfh", [128, NG * FEAT], BF16)
    bf_d = dram_in("bfh", [128, NF], F32)
    wp1_d = dram_in("wp1h", [128, NF * FH], BF16)
    bp1_d = dram_in("bp1h", [128, NH], F32)
    wp2_d = dram_in("wp2h", [128, NH * FH], BF16)
    bp2_d = dram_in("bp2h", [128, NH], F32)
    wp3_d = dram_in("wp3h", [128, NH * 2], BF16)
    bp3_d = dram_in("bp3h", [2, 1], F32)
    npad_d = dram_in("npadv", [128, 1], F32)

    hfT_o = nc.dram_tensor("hfT", [FEAT, GTPAD], F32, kind="ExternalOutput")
    pred_o = nc.dram_tensor("predT", [2, GTPAD], F32, kind="ExternalOutput")
    if debug_hb:
        dbg_o = nc.dram_tensor("dbg_hb", [P, D], BF16, kind="ExternalOutput")
    if debug_y2:
        dbgy_o = nc.dram_tensor("dbg_y2", [NG, 128, P], BF16,
                                kind="ExternalOutput")
    if debug_aggr:
        dbga_o = nc.dram_tensor("dbg_aggr", [NG, 128, P], BF16,
                                kind="ExternalOutput")

    RG = [list(range(NC))]

    def _mi(x):
        return getattr(x, "ins", x)

    _ring = {}

    def war_deps(tag, bufs, writer_inst):
        # no-op: dep tracking is sound now that skip_group_check (which
        # silently dropped instruction access registration) is unused
        pass

    def war_record(tag, readers):
        pass

    with tile.TileContext(nc) as tc:
        with tc.tile_pool(name="dram", bufs=1, space="DRAM") as dram, \
             tc.tile_pool(name="res", bufs=1) as res, \
             tc.tile_pool(name="wp", bufs=1) as wp, \
             tc.tile_pool(name="stream", bufs=3) as strm, \
             tc.tile_pool(name="work", bufs=2) as work, \
             tc.tile_pool(name="ps_a", bufs=2, space="PSUM") as ps_a, \
             tc.tile_pool(name="ps_b", bufs=2, space="PSUM") as ps_b, \
             tc.tile_pool(name="ps_c", bufs=2, space="PSUM") as ps_c, \
             tc.tile_pool(name="ps_t", bufs=2, space="PSUM") as ps_t:

            hb = dram.tile([P, DP], BF16, tag="hb")
            hglob = dram.tile([NC * P, DP], BF16, tag="hglob")
            bn_in = dram.tile([128, 2 * NG], F32, tag="bn_in")
            bn_out = dram.tile([128, 2 * NG], F32, tag="bn_out")

            # residents
            hT = [res.tile([128, P], BF16, tag=f"hT{g}", name=f"hT{g}") for g in range(NG)]
            gidx_t = res.tile([128, TOTCH * 8], mybir.dt.int16, tag="gidx")
            nc.sync.dma_start(gidx_t[:], gidx_d[:])
            t0_t = res.tile([16, KD], BF16, tag="t0")
            nc.sync.dma_start(t0_t[:], t0_d[:])
            npad_t = res.tile([128, 1], F32, tag="npad")
            nc.sync.dma_start(npad_t[:], npad_d[:])
            sacc = [res.tile([128, NST], F32, tag=f"sacc{g}", name=f"sacc{g}") for g in range(NG)]
            qacc = [res.tile([128, NST], F32, tag=f"qacc{g}", name=f"qacc{g}") for g in range(NG)]
            bnpack = res.tile([128, 2 * NG], F32, tag="bnpack")
            bnred = res.tile([128, 2 * NG], F32, tag="bnred")
            id_bf = res.tile([128, 128], BF16, tag="idbf")
            make_identity(nc, id_bf[:])
            zlhs = res.tile([16, 128], BF16, tag="zlhs")
            nc.vector.memset(zlhs[:], 0.0)

            def stsl(st):
                return slice(st * 512, (st + 1) * 512)

            # ---------------- h0 ----------------
            for st in range(NST):
                one_s = strm.tile([16, 512], BF16, tag="one_s")
                nc.sync.dma_start(one_s[:], one_d[:, stsl(st)])
                for g in range(NG):
                    ps = ps_a.tile([128, 512], F32, tag="paggr")
                    mm0 = nc.tensor.matmul(out=ps[:],
                                           lhsT=t0_t[:, g * 128:(g + 1) * 128],
                                           rhs=one_s[:], start=True, stop=True)
                    war_deps("paggr", 2, mm0)
                    rd = nc.scalar.copy(hT[g][:, stsl(st)], ps[:])
                    war_record("paggr", [rd])
                nm = work.tile([128, 4 * D], BF16, tag="nm")
                for t4 in range(4):
                    pn = ps_t.tile([128, D], F32, tag="ptr", name="pnm")
                    mm0 = nc.tensor.matmul(
                        out=pn[:],
                        lhsT=one_s[:, t4 * 128:(t4 + 1) * 128],
                        rhs=t0_t[:, :D], start=True, stop=True)
                    war_deps("ptr", 2, mm0)
                    rd = nc.vector.tensor_copy(nm[:, t4 * D:(t4 + 1) * D], pn[:])
                    war_record("ptr", [rd])
                nc.sync.dma_start(
                    hb[stsl(st), :D].rearrange("(i p) d -> p i d", p=128),
                    nm[:].rearrange("p (i d) -> p i d", d=D))
            nc.gpsimd.collective_compute(
                "AllGather", ALU.bypass, replica_groups=RG,
                ins=[hb.opt()], outs=[hglob.opt()])

            # ---------------- layers ----------------
            for l in range(run_layers):
                w1t = wp.tile([128, NG * KD2], BF16, tag="w1")
                nc.sync.dma_start(w1t[:], w1_d[l])
                w2t = wp.tile([128, NM * KD], BF16, tag="w2")
                nc.sync.dma_start(w2t[:], w2_d[l])
                t15t = wp.tile([16, KD], BF16, tag="t15")
                nc.sync.dma_start(t15t[:], t15_d[l])
                b1t = wp.tile([128, NM], F32, tag="b1")
                nc.sync.dma_start(b1t[:], b1_d[l])
                b2t = wp.tile([128, NG], F32, tag="b2")
                nc.sync.dma_start(b2t[:], b2_d[l])
                gmt = wp.tile([128, NG], F32, tag="gm")
                nc.sync.dma_start(gmt[:], gm_d[l])
                btt = wp.tile([128, NG], F32, tag="bt")
                nc.sync.dma_start(btt[:], bt_d[l])

                for st in range(NST):
                    k4 = K4[st]
                    off = st_off[st]
                    cnt_s = strm.tile([16, 512], BF16, tag="cnt_s")
                    nc.sync.dma_start(cnt_s[:], cnt_d[:, stsl(st)])
                    sel_s = strm.tile([128, max(k4, 1) * 128], BF16, tag="sel_s", bufs=2)
                    if k4:
                        nc.sync.dma_start(sel_s[:, :k4 * 128],
                                          sel_d[:, off * 128:(off + k4) * 128])
                    gb = strm.tile([128, max(k4, 1) * DP], BF16, tag="gb", bufs=2)
                    cbase = 0
                    for w in range(NW):
                        span = st_wspan[st][w]
                        if span == 0:
                            continue
                        nidx = span * 128
                        nc.gpsimd.dma_gather(
                            out_ap=gb[:, cbase * DP:(cbase + span) * DP]
                            .rearrange("p (c d) -> p c d", d=DP),
                            in_ap=hglob[w * 2 * P:(w + 1) * 2 * P, :],
                            idxs_ap=gidx_t[:, (off + cbase) * 8:
                                           (off + cbase + span) * 8],
                            num_idxs=nidx, num_idxs_reg=nidx,
                            elem_size=DP, single_packet=False)
                        cbase += span
                    aggr = []
                    for g in range(NG):
                        ps = ps_a.tile([128, 512], F32, tag="paggr")
                        mm0 = nc.tensor.matmul(out=ps[:],
                                               lhsT=t15t[:, g * 128:(g + 1) * 128],
                                               rhs=cnt_s[:], start=True,
                                               stop=(k4 == 0))
                        war_deps("paggr", 2, mm0)
                        for ci in range(k4):
                            tl = chunks[off + ci][2]
                            nc.tensor.matmul(
                                out=ps[:dgw[g], tl * 128:(tl + 1) * 128],
                                lhsT=gb[:, ci * DP + g * 128:
                                        ci * DP + g * 128 + dgw[g]],
                                rhs=sel_s[:, ci * 128:(ci + 1) * 128],
                                start=False, stop=False)
                        if k4:
                            nc.tensor.matmul(
                                out=ps[:, 0:1], lhsT=zlhs[:],
                                rhs=cnt_s[:, 0:1], start=False, stop=True)
                        a_sb = work.tile([128, 512], BF16, tag=f"agg{g}")
                        rd = nc.vector.tensor_tensor(out=a_sb[:], in0=ps[:],
                                                     in1=hT[g][:, stsl(st)],
                                                     op=ALU.add)
                        war_record("paggr", [rd])
                        if debug_aggr and l == run_layers - 1:
                            nc.sync.dma_start(dbga_o[g][:, stsl(st)], a_sb[:])
                        aggr.append(a_sb)
                    y1 = []
                    for mg in range(NM):
                        psy = ps_b.tile([128, 512], F32, tag="py1")
                        for kc in range(NG):
                            mm = nc.tensor.matmul(
                                out=psy[:],
                                lhsT=w1t[:, kc * KD2 + mg * 128:
                                         kc * KD2 + (mg + 1) * 128],
                                rhs=aggr[kc][:], start=(kc == 0),
                                stop=(kc == NG - 1))
                            if kc == 0:
                                war_deps("py1", 2, mm)
                        y_sb = work.tile([128, 512], BF16, tag=f"y1_{mg}")
                        rd = nc.scalar.activation(y_sb[:], psy[:], AF.Relu,
                                                  bias=b1t[:, mg:mg + 1],
                                                  scale=1.0)
                        war_record("py1", [rd])
                        y1.append(y_sb)
                    for g in range(NG):
                        psy2 = ps_c.tile([128, 512], F32, tag="py2")
                        for kc in range(NM):
                            mm = nc.tensor.matmul(
                                out=psy2[:],
                                lhsT=w2t[:, kc * KD + g * 128:
                                         kc * KD + (g + 1) * 128],
                                rhs=y1[kc][:], start=(kc == 0),
                                stop=(kc == NM - 1))
                            if kc == 0:
                                war_deps("py2", 2, mm)
                        rd1 = nc.vector.tensor_copy(hT[g][:, stsl(st)], psy2[:])
                        scr = work.tile([128, GTPAD], F32, tag="scr", bufs=1, name="scr")
                        rd2 = nc.scalar.activation(scr[:, :512], psy2[:],
                                                   AF.Square,
                                                   accum_out=qacc[g][:, st:st + 1])
                        war_record("py2", [rd1, rd2])
                        nc.vector.tensor_reduce(
                            out=sacc[g][:, st:st + 1], in_=hT[g][:, stsl(st)],
                            axis=mybir.AxisListType.X, op=ALU.add)

                if debug_y2 and l == run_layers - 1:
                    for g in range(NG):
                        nc.sync.dma_start(dbgy_o[g], hT[g][:])

                # stats + AllReduce
                tiny = [work.tile([128, 1], F32, tag=f"tiny{i}", name=f"tiny{i}")
                        for i in range(4)]
                for g in range(NG):
                    nc.vector.tensor_reduce(out=bnpack[:, 2 * g:2 * g + 1],
                                            in_=sacc[g][:],
                                            axis=mybir.AxisListType.X,
                                            op=ALU.add)
                    nc.vector.tensor_reduce(out=bnpack[:, 2 * g + 1:2 * g + 2],
                                            in_=qacc[g][:],
                                            axis=mybir.AxisListType.X,
                                            op=ALU.add)
                    r0 = tiny[0]
                    nc.vector.tensor_copy(r0[:], hT[g][:, P - 1:P])
                    t1 = tiny[1]
                    nc.vector.tensor_tensor(out=t1[:], in0=npad_t[:], in1=r0[:],
                                            op=ALU.mult)
                    nc.vector.tensor_tensor(out=bnpack[:, 2 * g:2 * g + 1],
                                            in0=bnpack[:, 2 * g:2 * g + 1],
                                            in1=t1[:], op=ALU.subtract)
                    t2 = tiny[2]
                    nc.vector.tensor_tensor(out=t2[:], in0=r0[:], in1=r0[:],
                                            op=ALU.mult)
                    nc.vector.tensor_tensor(out=t2[:], in0=t2[:], in1=npad_t[:],
                                            op=ALU.mult)
                    nc.vector.tensor_tensor(out=bnpack[:, 2 * g + 1:2 * g + 2],
                                            in0=bnpack[:, 2 * g + 1:2 * g + 2],
                                            in1=t2[:], op=ALU.subtract)
                nc.sync.dma_start(bn_in[:], bnpack[:])
                nc.gpsimd.collective_compute(
                    "AllReduce", ALU.add, replica_groups=RG,
                    ins=[bn_in.opt()], outs=[bn_out.opt()])
                nc.sync.dma_start(bnred[:], bn_out[:])

                a_v, c_v = [], []
                for g in range(NG):
                    mu = work.tile([128, 1], F32, tag=f"mu{g}")
                    nc.scalar.mul(mu[:], bnred[:, 2 * g:2 * g + 1], 1.0 / Ntot)
                    ms = work.tile([128, 1], F32, tag=f"ms{g}")
                    nc.scalar.mul(ms[:], bnred[:, 2 * g + 1:2 * g + 2],
                                  1.0 / Ntot)
                    var = work.tile([128, 1], F32, tag=f"var{g}")
                    nc.vector.tensor_tensor(out=var[:], in0=mu[:], in1=mu[:],
                                            op=ALU.mult)
                    nc.vector.tensor_tensor(out=var[:], in0=ms[:], in1=var[:],
                                            op=ALU.subtract)
                    nc.vector.tensor_scalar_add(var[:], var[:], float(EPS))
                    sd = work.tile([128, 1], F32, tag=f"sd{g}")
                    nc.scalar.sqrt(sd[:], var[:])
                    inv = work.tile([128, 1], F32, tag=f"inv{g}")
                    nc.vector.reciprocal(inv[:], sd[:])
                    av = work.tile([128, 1], F32, tag=f"av{g}")
                    nc.vector.tensor_tensor(out=av[:], in0=inv[:],
                                            in1=gmt[:, g:g + 1], op=ALU.mult)
                    cv = work.tile([128, 1], F32, tag=f"cv{g}")
                    nc.vector.tensor_tensor(out=cv[:], in0=mu[:], in1=av[:],
                                            op=ALU.mult)
                    nc.vector.tensor_tensor(out=cv[:], in0=btt[:, g:g + 1],
                                            in1=cv[:], op=ALU.subtract)
                    a_v.append(av)
                    c_v.append(cv)

                fn = AF.Relu if l < L - 1 else AF.Identity
                for st in range(NST):
                    nm = work.tile([128, 4 * D], BF16, tag="nm")
                    for g in range(NG):
                        nc.scalar.activation(hT[g][:, stsl(st)],
                                             hT[g][:, stsl(st)], fn,
                                             bias=c_v[g][:], scale=a_v[g][:])
                    for t4 in range(4):
                        tc_ = st * 4 + t4
                        for g in range(NG):
                            pt = ps_t.tile([128, 128], BF16, tag="ptr")
                            mm0 = nc.tensor.transpose(
                                out=pt[:],
                                in_=hT[g][:, tc_ * 128:(tc_ + 1) * 128],
                                identity=id_bf[:])
                            war_deps("ptr", 2, mm0)
                            rd = nc.vector.tensor_copy(
                                nm[:, t4 * D + g * 128:
                                   t4 * D + g * 128 + dgw[g]],
                                pt[:, :dgw[g]])
                            war_record("ptr", [rd])
                    nc.sync.dma_start(
                        hb[stsl(st), :D].rearrange("(i p) d -> p i d", p=128),
                        nm[:].rearrange("p (i d) -> p i d", d=D))
                if l < L - 1:
                    nc.gpsimd.collective_compute(
                        "AllGather", ALU.bypass, replica_groups=RG,
                        ins=[hb.opt()], outs=[hglob.opt()])

            if debug_hb:
                dtile = work.tile([128, 4 * D], BF16, tag="nm", name="dtile")
                for st in range(NST):
                    nc.sync.dma_start(
                        dtile[:].rearrange("p (i d) -> p i d", d=D),
                        hb[stsl(st), :D].rearrange("(i p) d -> p i d", p=128))
                    nc.sync.dma_start(
                        dbg_o[stsl(st), :].rearrange("(i p) d -> p i d", p=128),
                        dtile[:].rearrange("p (i d) -> p i d", d=D))

            # ---------------- pooling ----------------
            pooledT = [res.tile([128, GTPAD], BF16, tag=f"pool{g}", name=f"poolT{g}")
                       for g in range(NG)]
            for g in range(NG):
                nc.vector.memset(pooledT[g][:], 0.0)
            woff = 0
            for gt in range(GT):
                Wg = pW[gt]
                if Wg == 0:
                    continue
                span = strm.tile([128, Wg * D], BF16, tag="gb", bufs=2, name="pspan")
                nc.sync.dma_start(
                    span[:].rearrange("p (i d) -> p i d", d=D),
                    hb[plo[gt] * 128:(plo[gt] + Wg) * 128, :D]
                    .rearrange("(i p) d -> p i d", p=128))
                pse = strm.tile([128, Wg * 128], BF16, tag="sel_s", bufs=2, name="psel")
                nc.sync.dma_start(pse[:],
                                  pool_d[:, woff * 128:(woff + Wg) * 128])
                for g in range(NG):
                    pp = ps_a.tile([128, 512], F32, tag="paggr", name="ppool")
                    for w in range(Wg):
                        mm = nc.tensor.matmul(
                            out=pp[:dgw[g], :128],
                            lhsT=span[:, w * D + g * 128:
                                      w * D + g * 128 + dgw[g]],
                            rhs=pse[:, w * 128:(w + 1) * 128],
                            start=(w == 0), stop=(w == Wg - 1))
                        if w == 0:
                            war_deps("paggr", 2, mm)
                    rd = nc.vector.tensor_copy(
                        pooledT[g][:dgw[g], gt * 128:(gt + 1) * 128],
                        pp[:dgw[g], :128])
                    war_record("paggr", [rd])
                woff += Wg

            # ---------------- head ----------------
            wf_t = res.tile([128, NG * FEAT], BF16, tag="wf")
            nc.sync.dma_start(wf_t[:], wf_d[:])
            bf_t = res.tile([128, NF], F32, tag="bf")
            nc.sync.dma_start(bf_t[:], bf_d[:])
            wp1t = res.tile([128, NF * FH], BF16, tag="wp1")
            nc.sync.dma_start(wp1t[:], wp1_d[:])
            bp1t = res.tile([128, NH], F32, tag="bp1")
            nc.sync.dma_start(bp1t[:], bp1_d[:])
            wp2t = res.tile([128, NH * FH], BF16, tag="wp2")
            nc.sync.dma_start(wp2t[:], wp2_d[:])
            bp2t = res.tile([128, NH], F32, tag="bp2")
            nc.sync.dma_start(bp2t[:], bp2_d[:])
            wp3t = res.tile([128, NH * 2], BF16, tag="wp3")
            nc.sync.dma_start(wp3t[:], wp3_d[:])
            bp3t = res.tile([2, 1], F32, tag="bp3")
            nc.sync.dma_start(bp3t[:], bp3_d[:])

            ghs = [(i * 512, min(512, GTPAD - i * 512))
                   for i in range((GTPAD + 511) // 512)]

            hfb = [res.tile([128, GTPAD], BF16, tag=f"hfb{m}", name=f"hfb{m}")
                   for m in range(NF)]
            for mg in range(NF):
                hff = work.tile([128, GTPAD], F32, tag="scr", bufs=1, name="hff")
                for g0_, nh in ghs:
                    psh = ps_b.tile([128, 512], F32, tag="py1")
                    for kc in range(NG):
                        mm = nc.tensor.matmul(
                            out=psh[:, :nh],
                            lhsT=wf_t[:, kc * FEAT + mg * 128:
                                      kc * FEAT + (mg + 1) * 128],
                            rhs=pooledT[kc][:, g0_:g0_ + nh],
                            start=(kc == 0), stop=(kc == NG - 1))
                        if kc == 0:
                            war_deps("py1", 2, mm)
                    rd = nc.scalar.activation(hff[:, g0_:g0_ + nh], psh[:, :nh],
                                              AF.Identity,
                                              bias=bf_t[:, mg:mg + 1],
                                              scale=1.0)
                    war_record("py1", [rd])
                nc.vector.tensor_copy(hfb[mg][:], hff[:])
                nc.sync.dma_start(hfT_o[mg * 128:(mg + 1) * 128, :], hff[:])

            p1 = [res.tile([128, GTPAD], BF16, tag=f"p1_{m}", name=f"p1_{m}")
                  for m in range(NH)]
            for m2 in range(NH):
                for g0_, nh in ghs:
                    psh = ps_b.tile([128, 512], F32, tag="py1")
                    for kc in range(NF):
                        mm = nc.tensor.matmul(
                            out=psh[:, :nh],
                            lhsT=wp1t[:, kc * FH + m2 * 128:
                                      kc * FH + (m2 + 1) * 128],
                            rhs=hfb[kc][:, g0_:g0_ + nh],
                            start=(kc == 0), stop=(kc == NF - 1))
                        if kc == 0:
                            war_deps("py1", 2, mm)
                    spe = work.tile([128, 512], F32, tag="spe", name="spe", bufs=1)
                    rd = nc.scalar.activation(spe[:, :nh], psh[:, :nh], AF.Exp,
                                              bias=bp1t[:, m2:m2 + 1],
                                              scale=1.0)
                    war_record("py1", [rd])
                    nc.vector.tensor_scalar_add(spe[:, :nh], spe[:, :nh], 1.0)
                    nc.scalar.activation(p1[m2][:, g0_:g0_ + nh], spe[:, :nh],
                                         AF.Ln)
            p2 = [res.tile([128, GTPAD], BF16, tag=f"p2_{m}", name=f"p2_{m}")
                  for m in range(NH)]
            for m2 in range(NH):
                for g0_, nh in ghs:
                    psh = ps_b.tile([128, 512], F32, tag="py1")
                    for kc in range(NH):
                        mm = nc.tensor.matmul(
                            out=psh[:, :nh],
                            lhsT=wp2t[:, kc * FH + m2 * 128:
                                      kc * FH + (m2 + 1) * 128],
                            rhs=p1[kc][:, g0_:g0_ + nh],
                            start=(kc == 0), stop=(kc == NH - 1))
                        if kc == 0:
                            war_deps("py1", 2, mm)
                    spe = work.tile([128, 512], F32, tag="spe", name="spe", bufs=1)
                    rd = nc.scalar.activation(spe[:, :nh], psh[:, :nh], AF.Exp,
                                              bias=bp2t[:, m2:m2 + 1],
                                              scale=1.0)
                    war_record("py1", [rd])
                    nc.vector.tensor_scalar_add(spe[:, :nh], spe[:, :nh], 1.0)
                    nc.scalar.activation(p2[m2][:, g0_:g0_ + nh], spe[:, :nh],
                                         AF.Ln)
            prf = work.tile([2, GTPAD], F32, tag="prf", bufs=1)
            for g0_, nh in ghs:
                psd = ps_c.tile([128, 512], F32, tag="py2", name="ppred")
                for kc in range(NH):
                    mm = nc.tensor.matmul(
                        out=psd[:2, :nh],
                        lhsT=wp3t[:, kc * 2:(kc + 1) * 2],
                        rhs=p2[kc][:, g0_:g0_ + nh],
                        start=(kc == 0), stop=(kc == NH - 1))
                    if kc == 0:
                        war_deps("py2", 2, mm)
                rd = nc.scalar.activation(prf[:, g0_:g0_ + nh], psd[:2, :nh],
                                          AF.Identity, bias=bp3t[:], scale=1.0)
                war_record("py2", [rd])
            nc.sync.dma_start(pred_o[:], prf[:])

    return nc


# --------------------------------------------------------------------------
# host wrapper
# --------------------------------------------------------------------------

def make_in_maps(S, devs, pl):
    L = pl["t15h"].shape[0]
    maps = []
    for d in range(S["ncores"]):
        dv = devs[d]
        # wrapped-16 int16 idx layout, replicated across the 8 Q7 core blocks:
        # slot j of chunk ch at [j % 16, ch * 8 + j // 16]
        g16 = np.zeros((16, S["TOTCH"] * 8), np.int16)
        gi = dv["gidx"]                  # [128, TOTCH] window-local rows
        for ch in range(S["TOTCH"]):
            w16 = gi[:, ch].reshape(8, 16).T    # j at [j%16, j//16]
            g16[:, ch * 8:(ch + 1) * 8] = w16
        gidx16 = np.tile(g16, (8, 1))
        m = dict(
            gidx16=gidx16,
            selTh=np.ascontiguousarray(
                dv["selT"].transpose(1, 0, 2).reshape(128, -1)).astype(BF),
            cntT=dv["cntT"].astype(BF),
            oneT=dv["oneT"].astype(BF),
            poolh=np.ascontiguousarray(
                dv["poolsel"].transpose(1, 0, 2).reshape(128, -1)).astype(BF),
            t0h=pl["t0h"].astype(BF),
            t15h=pl["t15h"].astype(BF),
            w1h=pl["w1h"].astype(BF),
            w2h=pl["w2h"].astype(BF),
            b1h=pl["b1h"].astype(np.float32),
            b2h=pl["b2h"].astype(np.float32),
            gmh=pl["gmh"].astype(np.float32),
            bth=pl["bth"].astype(np.float32),
            wfh=pl["wfh"].astype(BF),
            bfh=pl["bfh"].astype(np.float32),
            wp1h=pl["wp1h"].astype(BF),
            bp1h=pl["bp1h"].astype(np.float32),
            wp2h=pl["wp2h"].astype(BF),
            bp2h=pl["bp2h"].astype(np.float32),
            wp3h=pl["wp3h"].astype(BF),
            bp3h=pl["bp3h"].astype(np.float32),
            npadv=np.full((128, 1), S["pshard"] - dv["nr"], np.float32),
        )
        maps.append(m)
    return maps


_CACHE = {}


def kernel(**inputs):
    x = np.asarray(inputs["x"])
    edge_index = np.asarray(inputs["edge_index"])
    edge_attr = np.asarray(inputs["edge_attr"])
    batch = np.asarray(inputs["batch"])
    B = 4096
    L, D, FEAT = 5, 300, 512

    S, devs = make_plan(x, edge_index, edge_attr, batch, B)
    pl = layout_params(inputs, L, D, FEAT)
    in_maps = make_in_maps(S, devs, pl)

    key = (S["pshard"], S["TOTCH"], tuple(S["K4"]), S["gtpad"],
           tuple(S["pool_W"]), tuple(S["pool_lo"]))
    if key not in _CACHE:
        nc = bacc.Bacc("TRN2", target_bir_lowering=False, debug=False,
                       num_devices=NCORES)
        build_gnn(nc, S, L, D, FEAT)
        nc.compile()
        _CACHE[key] = nc
    nc = _CACHE[key]

    res = run_bass_kernel_spmd(nc, in_maps, core_ids=list(range(NCORES)))

    hf = np.zeros((B, FEAT), np.float32)
    pred = np.zeros((B, 2), np.float32)
    for d in range(NCORES):
        dv = devs[d]
        g0, gr = dv["g0"], dv["gr"]
        hf[g0:g0 + gr] = res.results[d]["hfT"].T[:gr]
        pred[g0:g0 + gr] = res.results[d]["predT"].T[:gr]
    return hf, pred


# revision 21
# speedup vs baseline: 1.1226x; 1.1226x over previous
"""Trainium2 Bass kernel for nn_AGILE_7696581394505 (gnn_message_passing).

5-layer GIN-style message passing over N=100k nodes / E=200k edges,
BN per layer, mean-pool over B=4096 graphs, MLP head.
Distributed over 8 NeuronCores: nodes sharded contiguously at graph
boundaries; per-layer h AllGather; BN stats AllReduce.

Device algorithm (all matmuls on TensorE, D-on-partition "transposed" MLP):
  aggr^T = table15_l^T-slices @ counts^T  (edge-embedding sums, K=16 matmul)
         + sum_chunks  M_chunk^T @ S_chunk  (gathered h[src] rows x 0/1 selector)
         + h^T (self-loop)
  y1^T = relu(W1^T-slices @ aggr^T + b1)   y2^T = W2^T @ y1^T + b2
  BN via free-axis sums + tiny AllReduce (+ pad-row correction), normalize as
  one scalar.activation (scale=a, bias=c, Relu fused).
  h^T -> PE-transpose -> node-major -> DRAM -> AllGather (next layer's gather).

kernel(**inputs) -> (hf [4096,512] f32, pred [4096,2] f32)
"""
import numpy as np
import ml_dtypes

import concourse.bass as bass
import concourse.mybir as mybir
from concourse import bacc
import concourse.tile as tile
from concourse.bass_utils import run_bass_kernel_spmd
from concourse.masks import make_identity
from concourse.tile import add_dep_helper

F32 = mybir.dt.float32
BF16 = mybir.dt.bfloat16
I32 = mybir.dt.int32
AF = mybir.ActivationFunctionType
ALU = mybir.AluOpType
BF = ml_dtypes.bfloat16

EPS = 1e-5
NCORES = 8


# --------------------------------------------------------------------------
# host preprocessing
# --------------------------------------------------------------------------

def make_plan(x, edge_index, edge_attr, batch, B, ncores=NCORES):
    N = x.shape[0]
    gcnt = np.bincount(batch, minlength=B)
    gcum = np.concatenate([[0], np.cumsum(gcnt)])
    gbound = [0]
    for d in range(1, ncores):
        target = N * d / ncores
        g = int(np.searchsorted(gcum, target))
        if g > 0 and abs(gcum[g - 1] - target) < abs(gcum[g] - target):
            g -= 1
        g = max(gbound[-1], min(g, B))
        gbound.append(g)
    gbound.append(B)
    nbound = [int(gcum[g]) for g in gbound]
    nreal = [nbound[d + 1] - nbound[d] for d in range(ncores)]
    greal = [gbound[d + 1] - gbound[d] for d in range(ncores)]

    m = max(nreal)
    pshard = ((m + 255) // 512 + 1) * 512          # mult of 512, >=128 pad slack
    NT = pshard // 128
    NST = NT // 4
    gtpad = ((max(greal) + 127) // 128 + 1) * 128
    GT = gtpad // 128

    owner = np.zeros(N, np.int32)
    gslot = np.zeros(N, np.int64)
    tile_of = np.zeros(N, np.int64)
    for d in range(ncores):
        sl = slice(nbound[d], nbound[d + 1])
        owner[sl] = d
        gslot[sl] = np.arange(nreal[d]) + d * pshard
        tile_of[sl] = np.arange(nreal[d]) // 128

    combo_e = (edge_attr[:, 0] * 3 + edge_attr[:, 1]).astype(np.int64)
    combo_x = (x[:, 0] * 3 + x[:, 1]).astype(np.int64)
    src, dst = edge_index[0].astype(np.int64), edge_index[1].astype(np.int64)
    edst_dev = owner[dst]

    NW = ncores // 2                      # gather source windows (2 shards each)
    NPAIR = NT // 2                       # chunk dst width = 256 (tile pair)
    win_of = owner // 2                   # window of a node's owner
    pair_of = tile_of // 2
    cnt_wtd = np.zeros((ncores, NW, NPAIR), np.int64)
    np.add.at(cnt_wtd, (edst_dev, win_of[src], pair_of[dst]), 1)
    K_wt = np.ceil(cnt_wtd / 128).max(axis=0).astype(np.int64)   # [NW, NPAIR]

    chunks = []     # (st, w, pair_local in 0..1) per chunk; per st: w-major
    st_off = []
    st_wspan = []   # [NST][NW] chunk count
    for st in range(NST):
        st_off.append(len(chunks))
        spans = []
        for w in range(NW):
            n0c = len(chunks)
            for tl in range(2):
                for _ in range(int(K_wt[w, st * 2 + tl])):
                    chunks.append((st, w, tl))
            spans.append(len(chunks) - n0c)
        st_wspan.append(spans)
    TOTCH = len(chunks)
    st_off.append(TOTCH)
    K4 = [st_off[i + 1] - st_off[i] for i in range(NST)]

    devs = []
    for d in range(ncores):
        n0, n1 = nbound[d], nbound[d + 1]
        nr = n1 - n0
        mask = edst_dev == d
        es, ed, ec = src[mask], dst[mask], combo_e[mask]
        ew = win_of[es]
        order = np.lexsort((ed - n0, ew))    # by (window, dst)
        es, ed, ec, ew = es[order], ed[order], ec[order], ew[order]
        ld = ed - n0
        lt = ld // 128

        # slot indices per (window, tile): gidx16 holds window-local rows
        gidx = np.zeros((128, TOTCH), np.int32)   # window-local src row
        selT = np.zeros((TOTCH, 128, 256), np.float32)
        cntT = np.zeros((16, pshard), np.float32)
        np.add.at(cntT, (ec, ld), 1.0)
        cntT[12, :nr] += 1.0
        oneT = np.zeros((16, pshard), np.float32)
        oneT[combo_x[n0:n1], np.arange(nr)] = 1.0

        lp = lt // 2
        key = ew * NPAIR + lp
        kstart = np.searchsorted(key, np.arange(NW * NPAIR + 1))
        ch_of_wt = {}
        for i, (st, w, tl) in enumerate(chunks):
            ch_of_wt.setdefault((w, st * 2 + tl), []).append(i)
        for w in range(NW):
            for t in range(NPAIR):
                e0 = int(kstart[w * NPAIR + t])
                e1 = int(kstart[w * NPAIR + t + 1])
                if e1 == e0:
                    continue
                for k, ch in enumerate(ch_of_wt.get((w, t), [])):
                    a = e0 + k * 128
                    b = min(e0 + (k + 1) * 128, e1)
                    if a >= b:
                        break
                    lanes = np.arange(b - a)
                    gidx[lanes, ch] = (gslot[es[a:b]] - w * 2 * pshard).astype(
                        np.int32)
                    selT[ch, lanes, ld[a:b] - t * 256] = 1.0
        devs.append(dict(n0=n0, n1=n1, g0=gbound[d], g1=gbound[d + 1],
                         nr=nr, gr=greal[d], gidx=gidx, selT=selT,
                         cntT=cntT, oneT=oneT))

    # pooling bounds (compile-time shared)
    lo = [NT] * GT
    hi = [0] * GT
    for d in range(ncores):
        dv = devs[d]
        for gt in range(GT):
            ga = dv["g0"] + gt * 128
            gb = min(dv["g0"] + (gt + 1) * 128, dv["g1"])
            if ga >= dv["g1"]:
                continue
            na = int(gcum[ga]) - dv["n0"]
            nb = int(gcum[gb]) - dv["n0"]
            lo[gt] = min(lo[gt], na // 128)
            hi[gt] = max(hi[gt], (nb + 127) // 128)
    for gt in range(GT):
        if lo[gt] > hi[gt]:
            lo[gt], hi[gt] = 0, 0
    W = [hi[g] - lo[g] for g in range(GT)]
    SW = sum(W)

    for d in range(ncores):
        dv = devs[d]
        psel = np.zeros((max(SW, 1), 128, 128), np.float32)
        off = 0
        for gt in range(GT):
            for w in range(W[gt]):
                ncki = lo[gt] + w
                nloc = np.arange(ncki * 128, (ncki + 1) * 128)
                valid = nloc < dv["nr"]
                gl = np.full(128, -1, np.int64)
                nglob = dv["n0"] + nloc[valid]
                gl[valid] = batch[nglob] - dv["g0"] - gt * 128
                sel = (gl >= 0) & (gl < 128)
                lanes = np.arange(128)[sel]
                gcol = gl[sel]
                gglob = batch[dv["n0"] + nloc[sel]]
                psel[off + w, lanes, gcol] = 1.0 / np.maximum(gcnt[gglob], 1)
            off += W[gt]
        dv["poolsel"] = psel

    S = dict(N=N, B=B, ncores=ncores, pshard=pshard, NT=NT, NST=NST,
             gtpad=gtpad, GT=GT, TOTCH=TOTCH, K4=K4, st_off=st_off,
             chunks=chunks, NW=NW, st_wspan=st_wspan,
             pool_lo=lo, pool_W=W, SW=max(SW, 1))
    return S, devs


def make_tables(p, L, D):
    table0 = np.zeros((16, D), np.float32)
    for c in range(9):
        table0[c] = p["x_emb1"][c // 3] + p["x_emb2"][c % 3]
    table15 = np.zeros((L, 16, D), np.float32)
    for l in range(L):
        for c in range(15):
            table15[l, c] = p["edge_emb1"][l, c // 3] + p["edge_emb2"][l, c % 3]
    return table0, table15


def _lhsT_pack(Wm, KP, MP):
    """[K, M] weight -> [128, KP*MP] padded lhsT layout.
    col kc*MP + m holds Wm[kc*128 + kk, m] at partition kk."""
    K, M = Wm.shape
    nk = KP // 128 if KP % 128 == 0 else None
    nk = (KP + 127) // 128
    out = np.zeros((128, nk * MP), np.float32)
    for kc in range(nk):
        kk = min(128, K - kc * 128)
        if kk <= 0:
            break
        out[:kk, kc * MP:kc * MP + M] = Wm[kc * 128:kc * 128 + kk, :]
    return out


def _bias_pack(b, ngrp):
    out = np.zeros((128, ngrp), np.float32)
    for g in range(ngrp):
        n = min(128, len(b) - g * 128)
        if n <= 0:
            break
        out[:n, g] = b[g * 128:g * 128 + n]
    return out


def layout_params(p, L, D, FEAT):
    """Pack params into device layouts (f32; cast where needed later)."""
    D2 = 2 * D
    FH = FEAT // 2
    table0, table15 = make_tables(p, L, D)
    KD = ((D + 127) // 128) * 128       # 384
    KD2 = ((D2 + 127) // 128) * 128     # 640
    out = {}
    out["t0h"] = np.zeros((16, KD), np.float32)
    out["t0h"][:, :D] = table0
    out["t15h"] = np.zeros((L, 16, KD), np.float32)
    out["t15h"][:, :, :D] = table15
    out["w1h"] = np.stack([_lhsT_pack(np.asarray(p["W1"][l]), KD, KD2)
                           for l in range(L)])       # [L,128,3*640]
    out["w2h"] = np.stack([_lhsT_pack(np.asarray(p["W2"][l]), KD2, KD)
                           for l in range(L)])       # [L,128,5*384]
    out["b1h"] = np.stack([_bias_pack(np.asarray(p["b1"][l]), KD2 // 128)
                           for l in range(L)])
    out["b2h"] = np.stack([_bias_pack(np.asarray(p["b2"][l]), KD // 128)
                           for l in range(L)])
    out["gmh"] = np.stack([_bias_pack(np.asarray(p["gamma"][l]), KD // 128)
                           for l in range(L)])
    out["bth"] = np.stack([_bias_pack(np.asarray(p["beta"][l]), KD // 128)
                           for l in range(L)])
    out["wfh"] = _lhsT_pack(np.asarray(p["W_feat"]), KD, FEAT)   # [128, 3*512]
    out["bfh"] = _bias_pack(np.asarray(p["b_feat"]), FEAT // 128)
    out["wp1h"] = _lhsT_pack(np.asarray(p["Wp1"]), FEAT, FH)     # [128, 4*256]
    out["bp1h"] = _bias_pack(np.asarray(p["bp1"]), FH // 128)
    out["wp2h"] = _lhsT_pack(np.asarray(p["Wp2"]), FH, FH)       # [128, 2*256]
    out["bp2h"] = _bias_pack(np.asarray(p["bp2"]), FH // 128)
    out["wp3h"] = _lhsT_pack(np.asarray(p["Wp3"]), FH, 2)        # [128, 2*2]
    out["bp3h"] = np.asarray(p["bp3"], np.float32).reshape(2, 1)
    return out


# --------------------------------------------------------------------------
# device program
# --------------------------------------------------------------------------

def build_gnn(nc, S, L=5, D=300, FEAT=512, run_layers=None, debug_hb=False,
              debug_y2=False, debug_aggr=False, no_collectives=False):
    if run_layers is None:
        run_layers = L
    P = S["pshard"]
    NST, NT, GT = S["NST"], S["NT"], S["GT"]
    GTPAD = S["gtpad"]
    TOTCH, K4, st_off = S["TOTCH"], S["K4"], S["st_off"]
    chunks = S["chunks"]
    plo, pW, SW = S["pool_lo"], S["pool_W"], S["SW"]
    Ntot = S["N"]
    NC = S["ncores"]
    D2 = 2 * D
    FH = FEAT // 2
    DP = ((D + 191) // 128) * 128  # h row padded to 256B mult (384 for D=300)
    NW = S["NW"]
    st_wspan = S["st_wspan"]
    NG = (D + 127) // 128          # 3 channel groups
    NM = (D2 + 127) // 128         # 5 groups for y1
    NF = FEAT // 128               # 4
    NH = FH // 128                 # 2
    KD, KD2 = NG * 128, NM * 128
    dgw = [min(128, D - g * 128) for g in range(NG)]   # [128,128,44]

    # ---- dram I/O ----
    din = {}
    def dram_in(name, shape, dt):
        din[name] = nc.dram_tensor(name, shape, dt, kind="ExternalInput")
        return din[name]

    gidx_d = dram_in("gidx16", [128, TOTCH * 8], mybir.dt.int16)
    sel_d = dram_in("selTh", [128, TOTCH * 256], BF16)
    cnt_d = dram_in("cntT", [16, P], BF16)
    one_d = dram_in("oneT", [16, P], BF16)
    pool_d = dram_in("poolh", [128, SW * 128], BF16)
    t0_d = dram_in("t0h", [16, KD], BF16)
    t15_d = dram_in("t15h", [L, 16, KD], BF16)
    w1_d = dram_in("w1h", [L, 128, NG * KD2], BF16)
    w2_d = dram_in("w2h", [L, 128, NM * KD], BF16)
    b1_d = dram_in("b1h", [L, 128, NM], F32)
    b2_d = dram_in("b2h", [L, 128, NG], F32)
    gm_d = dram_in("gmh", [L, 128, NG], F32)
    bt_d = dram_in("bth", [L, 128, NG], F32)
    wf_d = dram_in("wfh", [128, NG * FEAT], BF16)
    bf_d = dram_in("bfh", [128, NF], F32)
    wp1_d = dram_in("wp1h", [128, NF * FH], BF16)
    bp1_d = dram_in("bp1h", [128, NH], F32)
    wp2_d = dram_in("wp2h", [128, NH * FH], BF16)
    bp2_d = dram_in("bp2h", [128, NH], F32)
    wp3_d = dram_in("wp3h", [128, NH * 2], BF16)
    bp3_d = dram_in("bp3h", [2, 1], F32)
    npad_d = dram_in("npadv", [128, 1], F32)

    hfT_o = nc.dram_tensor("hfT", [FEAT, GTPAD], F32, kind="ExternalOutput")
    pred_o = nc.dram_tensor("predT", [2, GTPAD], F32, kind="ExternalOutput")
    if debug_hb:
        dbg_o = nc.dram_tensor("dbg_hb", [P, D], BF16, kind="ExternalOutput")
    if debug_y2:
        dbgy_o = nc.dram_tensor("dbg_y2", [NG, 128, P], BF16,
                                kind="ExternalOutput")
    if debug_aggr:
        dbga_o = nc.dram_tensor("dbg_aggr", [NG, 128, P], BF16,
                                kind="ExternalOutput")

    RG = [list(range(NC))]

    def _mi(x):
        return getattr(x, "ins", x)

    _ring = {}

    def war_deps(tag, bufs, writer_inst):
        # no-op: dep tracking is sound now that skip_group_check (which
        # silently dropped instruction access registration) is unused
        pass

    def war_record(tag, readers):
        pass

    with tile.TileContext(nc) as tc:
        with tc.tile_pool(name="dram", bufs=1, space="DRAM") as dram, \
             tc.tile_pool(name="res", bufs=1) as res, \
             tc.tile_pool(name="wp", bufs=1) as wp, \
             tc.tile_pool(name="stream", bufs=3) as strm, \
             tc.tile_pool(name="work", bufs=2) as work, \
             tc.tile_pool(name="ps_a", bufs=2, space="PSUM") as ps_a, \
             tc.tile_pool(name="ps_b", bufs=2, space="PSUM") as ps_b, \
             tc.tile_pool(name="ps_c", bufs=2, space="PSUM") as ps_c, \
             tc.tile_pool(name="ps_t", bufs=2, space="PSUM") as ps_t:

            hb = dram.tile([P, DP], BF16, tag="hb")
            hglob = dram.tile([NC * P, DP], BF16, tag="hglob")
            bn_in = dram.tile([128, 2 * NG], F32, tag="bn_in")
            bn_out = dram.tile([128, 2 * NG], F32, tag="bn_out")

            # residents
            hT = [res.tile([128, P], BF16, tag=f"hT{g}", name=f"hT{g}") for g in range(NG)]
            gidx_t = res.tile([128, TOTCH * 8], mybir.dt.int16, tag="gidx")
            nc.sync.dma_start(gidx_t[:], gidx_d[:])
            t0_t = res.tile([16, KD], BF16, tag="t0")
            nc.sync.dma_start(t0_t[:], t0_d[:])
            npad_t = res.tile([128, 1], F32, tag="npad")
            nc.sync.dma_start(npad_t[:], npad_d[:])
            sacc = [res.tile([128, NST], F32, tag=f"sacc{g}", name=f"sacc{g}") for g in range(NG)]
            qacc = [res.tile([128, NST], F32, tag=f"qacc{g}", name=f"qacc{g}") for g in range(NG)]
            bnpack = res.tile([128, 2 * NG], F32, tag="bnpack")
            bnred = res.tile([128, 2 * NG], F32, tag="bnred")
            id_bf = res.tile([128, 128], BF16, tag="idbf")
            make_identity(nc, id_bf[:])
            zlhs = res.tile([16, 128], BF16, tag="zlhs")
            nc.vector.memset(zlhs[:], 0.0)

            def stsl(st):
                return slice(st * 512, (st + 1) * 512)

            # ---------------- h0 ----------------
            for st in range(NST):
                one_s = strm.tile([16, 512], BF16, tag="one_s")
                nc.sync.dma_start(one_s[:], one_d[:, stsl(st)])
                for g in range(NG):
                    ps = ps_a.tile([128, 512], F32, tag="paggr")
                    mm0 = nc.tensor.matmul(out=ps[:],
                                           lhsT=t0_t[:, g * 128:(g + 1) * 128],
                                           rhs=one_s[:], start=True, stop=True)
                    war_deps("paggr", 2, mm0)
                    rd = nc.scalar.copy(hT[g][:, stsl(st)], ps[:])
                    war_record("paggr", [rd])
                nm = work.tile([128, 4 * D], BF16, tag="nm")
                for t4 in range(4):
                    pn = ps_t.tile([128, D], F32, tag="ptr", name="pnm")
                    mm0 = nc.tensor.matmul(
                        out=pn[:],
                        lhsT=one_s[:, t4 * 128:(t4 + 1) * 128],
                        rhs=t0_t[:, :D], start=True, stop=True)
                    war_deps("ptr", 2, mm0)
                    rd = nc.vector.tensor_copy(nm[:, t4 * D:(t4 + 1) * D], pn[:])
                    war_record("ptr", [rd])
                nc.sync.dma_start(
                    hb[stsl(st), :D].rearrange("(i p) d -> p i d", p=128),
                    nm[:].rearrange("p (i d) -> p i d", d=D))
            def allgather_h():
                if no_collectives:
                    nc.sync.dma_start(hglob[0:P, :], hb[:])
                else:
                    nc.gpsimd.collective_compute(
                        "AllGather", ALU.bypass, replica_groups=RG,
                        ins=[hb.opt()], outs=[hglob.opt()])

            allgather_h()

            # ---------------- layers ----------------
            for l in range(run_layers):
                w1t = wp.tile([128, NG * KD2], BF16, tag="w1")
                nc.sync.dma_start(w1t[:], w1_d[l])
                w2t = wp.tile([128, NM * KD], BF16, tag="w2")
                nc.sync.dma_start(w2t[:], w2_d[l])
                t15t = wp.tile([16, KD], BF16, tag="t15")
                nc.sync.dma_start(t15t[:], t15_d[l])
                b1t = wp.tile([128, NM], F32, tag="b1")
                nc.sync.dma_start(b1t[:], b1_d[l])
                b2t = wp.tile([128, NG], F32, tag="b2")
                nc.sync.dma_start(b2t[:], b2_d[l])
                gmt = wp.tile([128, NG], F32, tag="gm")
                nc.sync.dma_start(gmt[:], gm_d[l])
                btt = wp.tile([128, NG], F32, tag="bt")
                nc.sync.dma_start(btt[:], bt_d[l])

                for st in range(NST):
                    k4 = K4[st]
                    off = st_off[st]
                    cnt_s = strm.tile([16, 512], BF16, tag="cnt_s")
                    nc.sync.dma_start(cnt_s[:], cnt_d[:, stsl(st)])
                    sel_s = strm.tile([128, max(k4, 1) * 256], BF16, tag="sel_s", bufs=2)
                    if k4:
                        nc.sync.dma_start(sel_s[:, :k4 * 256],
                                          sel_d[:, off * 256:(off + k4) * 256])
                    gb = strm.tile([128, max(k4, 1) * DP], BF16, tag="gb", bufs=2)
                    cbase = 0
                    for w in range(NW):
                        span = st_wspan[st][w]
                        if span == 0:
                            continue
                        nidx = span * 128
                        nc.gpsimd.dma_gather(
                            out_ap=gb[:, cbase * DP:(cbase + span) * DP]
                            .rearrange("p (c d) -> p c d", d=DP),
                            in_ap=hglob[w * 2 * P:(w + 1) * 2 * P, :],
                            idxs_ap=gidx_t[:, (off + cbase) * 8:
                                           (off + cbase + span) * 8],
                            num_idxs=nidx, num_idxs_reg=nidx,
                            elem_size=DP, single_packet=False)
                        cbase += span
                    aggr = []
                    for g in range(NG):
                        ps = ps_a.tile([128, 512], F32, tag="paggr")
                        mm0 = nc.tensor.matmul(out=ps[:],
                                               lhsT=t15t[:, g * 128:(g + 1) * 128],
                                               rhs=cnt_s[:], start=True,
                                               stop=(k4 == 0))
                        war_deps("paggr", 2, mm0)
                        for ci in range(k4):
                            tl = chunks[off + ci][2]
                            nc.tensor.matmul(
                                out=ps[:dgw[g], tl * 256:(tl + 1) * 256],
                                lhsT=gb[:, ci * DP + g * 128:
                                        ci * DP + g * 128 + dgw[g]],
                                rhs=sel_s[:, ci * 256:(ci + 1) * 256],
                                start=False, stop=False)
                        if k4:
                            nc.tensor.matmul(
                                out=ps[:, 0:1], lhsT=zlhs[:],
                                rhs=cnt_s[:, 0:1], start=False, stop=True)
                        a_sb = work.tile([128, 512], BF16, tag=f"agg{g}")
                        rd = nc.vector.tensor_tensor(out=a_sb[:], in0=ps[:],
                                                     in1=hT[g][:, stsl(st)],
                                                     op=ALU.add)
                        war_record("paggr", [rd])
                        if debug_aggr and l == run_layers - 1:
                            nc.sync.dma_start(dbga_o[g][:, stsl(st)], a_sb[:])
                        aggr.append(a_sb)
                    y1 = []
                    for mg in range(NM):
                        psy = ps_b.tile([128, 512], F32, tag="py1")
                        for kc in range(NG):
                            mm = nc.tensor.matmul(
                                out=psy[:],
                                lhsT=w1t[:, kc * KD2 + mg * 128:
                                         kc * KD2 + (mg + 1) * 128],
                                rhs=aggr[kc][:], start=(kc == 0),
                                stop=(kc == NG - 1))
                            if kc == 0:
                                war_deps("py1", 2, mm)
                        y_sb = work.tile([128, 512], BF16, tag=f"y1_{mg}")
                        if mg < 3:
                            nc.scalar.activation(y_sb[:], psy[:], AF.Relu,
                                                 bias=b1t[:, mg:mg + 1],
                                                 scale=1.0)
                        else:
                            nc.vector.tensor_scalar(
                                out=y_sb[:], in0=psy[:],
                                scalar1=b1t[:, mg:mg + 1], scalar2=0.0,
                                op0=ALU.add, op1=ALU.max)
                        y1.append(y_sb)
                    for g in range(NG):
                        psy2 = ps_c.tile([128, 512], F32, tag="py2")
                        for kc in range(NM):
                            mm = nc.tensor.matmul(
                                out=psy2[:],
                                lhsT=w2t[:, kc * KD + g * 128:
                                         kc * KD + (g + 1) * 128],
                                rhs=y1[kc][:], start=(kc == 0),
                                stop=(kc == NM - 1))
                            if kc == 0:
                                war_deps("py2", 2, mm)
                        nc.scalar.activation(hT[g][:, stsl(st)], psy2[:],
                                             AF.Copy,
                                             accum_out=sacc[g][:, st:st + 1])
                        scr = work.tile([128, max(512, GTPAD)], F32, tag="scr", bufs=1, name="scr")
                        nc.scalar.activation(scr[:, :512], psy2[:],
                                             AF.Square,
                                             accum_out=qacc[g][:, st:st + 1])

                if debug_y2 and l == run_layers - 1:
                    for g in range(NG):
                        nc.sync.dma_start(dbgy_o[g], hT[g][:])

                # stats + AllReduce
                tiny = [work.tile([128, 1], F32, tag=f"tiny{i}", name=f"tiny{i}")
                        for i in range(4)]
                for g in range(NG):
                    nc.vector.tensor_reduce(out=bnpack[:, 2 * g:2 * g + 1],
                                            in_=sacc[g][:],
                                            axis=mybir.AxisListType.X,
                                            op=ALU.add)
                    nc.vector.tensor_reduce(out=bnpack[:, 2 * g + 1:2 * g + 2],
                                            in_=qacc[g][:],
                                            axis=mybir.AxisListType.X,
                                            op=ALU.add)
                    r0 = tiny[0]
                    nc.vector.tensor_copy(r0[:], hT[g][:, P - 1:P])
                    t1 = tiny[1]
                    nc.vector.tensor_tensor(out=t1[:], in0=npad_t[:], in1=r0[:],
                                            op=ALU.mult)
                    nc.vector.tensor_tensor(out=bnpack[:, 2 * g:2 * g + 1],
                                            in0=bnpack[:, 2 * g:2 * g + 1],
                                            in1=t1[:], op=ALU.subtract)
                    t2 = tiny[2]
                    nc.vector.tensor_tensor(out=t2[:], in0=r0[:], in1=r0[:],
                                            op=ALU.mult)
                    nc.vector.tensor_tensor(out=t2[:], in0=t2[:], in1=npad_t[:],
                                            op=ALU.mult)
                    nc.vector.tensor_tensor(out=bnpack[:, 2 * g + 1:2 * g + 2],
                                            in0=bnpack[:, 2 * g + 1:2 * g + 2],
                                            in1=t2[:], op=ALU.subtract)
                nc.sync.dma_start(bn_in[:], bnpack[:])
                if no_collectives:
                    nc.sync.dma_start(bn_out[:], bn_in[:])
                else:
                    nc.gpsimd.collective_compute(
                        "AllReduce", ALU.add, replica_groups=RG,
                        ins=[bn_in.opt()], outs=[bn_out.opt()])
                nc.sync.dma_start(bnred[:], bn_out[:])

                a_v, c_v = [], []
                for g in range(NG):
                    mu = work.tile([128, 1], F32, tag=f"mu{g}")
                    nc.scalar.mul(mu[:], bnred[:, 2 * g:2 * g + 1], 1.0 / Ntot)
                    ms = work.tile([128, 1], F32, tag=f"ms{g}")
                    nc.scalar.mul(ms[:], bnred[:, 2 * g + 1:2 * g + 2],
                                  1.0 / Ntot)
                    var = work.tile([128, 1], F32, tag=f"var{g}")
                    nc.vector.tensor_tensor(out=var[:], in0=mu[:], in1=mu[:],
                                            op=ALU.mult)
                    nc.vector.tensor_tensor(out=var[:], in0=ms[:], in1=var[:],
                                            op=ALU.subtract)
                    nc.vector.tensor_scalar_add(var[:], var[:], float(EPS))
                    sd = work.tile([128, 1], F32, tag=f"sd{g}")
                    nc.scalar.sqrt(sd[:], var[:])
                    inv = work.tile([128, 1], F32, tag=f"inv{g}")
                    nc.vector.reciprocal(inv[:], sd[:])
                    av = work.tile([128, 1], F32, tag=f"av{g}")
                    nc.vector.tensor_tensor(out=av[:], in0=inv[:],
                                            in1=gmt[:, g:g + 1], op=ALU.mult)
                    cv = work.tile([128, 1], F32, tag=f"cv{g}")
                    nc.vector.tensor_tensor(out=cv[:], in0=mu[:], in1=av[:],
                                            op=ALU.mult)
                    nc.vector.tensor_tensor(out=cv[:], in0=btt[:, g:g + 1],
                                            in1=cv[:], op=ALU.subtract)
                    a_v.append(av)
                    c_v.append(cv)

                fn = AF.Relu if l < L - 1 else AF.Identity
                for st in range(NST):
                    nm = work.tile([128, 4 * D], BF16, tag="nm")
                    for g in range(NG):
                        nc.scalar.activation(hT[g][:, stsl(st)],
                                             hT[g][:, stsl(st)], fn,
                                             bias=c_v[g][:], scale=a_v[g][:])
                    for t4 in range(4):
                        tc_ = st * 4 + t4
                        pt = ps_t.tile([128, 3 * 128], BF16, tag="ptr")
                        for g in range(NG):
                            nc.tensor.transpose(
                                out=pt[:, g * 128:(g + 1) * 128],
                                in_=hT[g][:, tc_ * 128:(tc_ + 1) * 128],
                                identity=id_bf[:])
                        nc.vector.tensor_copy(
                            nm[:, t4 * D:t4 * D + D], pt[:, :D])
                    nc.sync.dma_start(
                        hb[stsl(st), :D].rearrange("(i p) d -> p i d", p=128),
                        nm[:].rearrange("p (i d) -> p i d", d=D))
                if l < L - 1:
                    allgather_h()

            if debug_hb:
                dtile = work.tile([128, 4 * D], BF16, tag="nm", name="dtile")
                for st in range(NST):
                    nc.sync.dma_start(
                        dtile[:].rearrange("p (i d) -> p i d", d=D),
                        hb[stsl(st), :D].rearrange("(i p) d -> p i d", p=128))
                    nc.sync.dma_start(
                        dbg_o[stsl(st), :].rearrange("(i p) d -> p i d", p=128),
                        dtile[:].rearrange("p (i d) -> p i d", d=D))

            # ---------------- pooling ----------------
            pooledT = [res.tile([128, GTPAD], BF16, tag=f"pool{g}", name=f"poolT{g}")
                       for g in range(NG)]
            for g in range(NG):
                nc.vector.memset(pooledT[g][:], 0.0)
            woff = 0
            for gt in range(GT):
                Wg = pW[gt]
                if Wg == 0:
                    continue
                span = strm.tile([128, Wg * D], BF16, tag="gb", bufs=2, name="pspan")
                nc.sync.dma_start(
                    span[:].rearrange("p (i d) -> p i d", d=D),
                    hb[plo[gt] * 128:(plo[gt] + Wg) * 128, :D]
                    .rearrange("(i p) d -> p i d", p=128))
                pse = strm.tile([128, Wg * 128], BF16, tag="sel_s", bufs=2, name="psel")
                nc.sync.dma_start(pse[:],
                                  pool_d[:, woff * 128:(woff + Wg) * 128])
                for g in range(NG):
                    pp = ps_a.tile([128, 512], F32, tag="paggr", name="ppool")
                    for w in range(Wg):
                        mm = nc.tensor.matmul(
                            out=pp[:dgw[g], :128],
                            lhsT=span[:, w * D + g * 128:
                                      w * D + g * 128 + dgw[g]],
                            rhs=pse[:, w * 128:(w + 1) * 128],
                            start=(w == 0), stop=(w == Wg - 1))
                        if w == 0:
                            war_deps("paggr", 2, mm)
                    rd = nc.vector.tensor_copy(
                        pooledT[g][:dgw[g], gt * 128:(gt + 1) * 128],
                        pp[:dgw[g], :128])
                    war_record("paggr", [rd])
                woff += Wg

            # ---------------- head ----------------
            wf_t = res.tile([128, NG * FEAT], BF16, tag="wf")
            nc.sync.dma_start(wf_t[:], wf_d[:])
            bf_t = res.tile([128, NF], F32, tag="bf")
            nc.sync.dma_start(bf_t[:], bf_d[:])
            wp1t = res.tile([128, NF * FH], BF16, tag="wp1")
            nc.sync.dma_start(wp1t[:], wp1_d[:])
            bp1t = res.tile([128, NH], F32, tag="bp1")
            nc.sync.dma_start(bp1t[:], bp1_d[:])
            wp2t = res.tile([128, NH * FH], BF16, tag="wp2")
            nc.sync.dma_start(wp2t[:], wp2_d[:])
            bp2t = res.tile([128, NH], F32, tag="bp2")
            nc.sync.dma_start(bp2t[:], bp2_d[:])
            wp3t = res.tile([128, NH * 2], BF16, tag="wp3")
            nc.sync.dma_start(wp3t[:], wp3_d[:])
            bp3t = res.tile([2, 1], F32, tag="bp3")
            nc.sync.dma_start(bp3t[:], bp3_d[:])

            ghs = [(i * 512, min(512, GTPAD - i * 512))
                   for i in range((GTPAD + 511) // 512)]

            hfb = [res.tile([128, GTPAD], BF16, tag=f"hfb{m}", name=f"hfb{m}")
                   for m in range(NF)]
            for mg in range(NF):
                hff = work.tile([128, max(512, GTPAD)], F32, tag="scr", bufs=1, name="hff")
                for g0_, nh in ghs:
                    psh = ps_b.tile([128, 512], F32, tag="py1")
                    for kc in range(NG):
                        mm = nc.tensor.matmul(
                            out=psh[:, :nh],
                            lhsT=wf_t[:, kc * FEAT + mg * 128:
                                      kc * FEAT + (mg + 1) * 128],
                            rhs=pooledT[kc][:, g0_:g0_ + nh],
                            start=(kc == 0), stop=(kc == NG - 1))
                        if kc == 0:
                            war_deps("py1", 2, mm)
                    rd = nc.scalar.activation(hff[:, g0_:g0_ + nh], psh[:, :nh],
                                              AF.Identity,
                                              bias=bf_t[:, mg:mg + 1],
                                              scale=1.0)
                    war_record("py1", [rd])
                nc.vector.tensor_copy(hfb[mg][:], hff[:, :GTPAD])
                nc.sync.dma_start(hfT_o[mg * 128:(mg + 1) * 128, :], hff[:, :GTPAD])

            p1 = [res.tile([128, GTPAD], BF16, tag=f"p1_{m}", name=f"p1_{m}")
                  for m in range(NH)]
            for m2 in range(NH):
                for g0_, nh in ghs:
                    psh = ps_b.tile([128, 512], F32, tag="py1")
                    for kc in range(NF):
                        mm = nc.tensor.matmul(
                            out=psh[:, :nh],
                            lhsT=wp1t[:, kc * FH + m2 * 128:
                                      kc * FH + (m2 + 1) * 128],
                            rhs=hfb[kc][:, g0_:g0_ + nh],
                            start=(kc == 0), stop=(kc == NF - 1))
                        if kc == 0:
                            war_deps("py1", 2, mm)
                    spe = work.tile([128, 512], F32, tag="spe", name="spe", bufs=1)
                    rd = nc.scalar.activation(spe[:, :nh], psh[:, :nh], AF.Exp,
                                              bias=bp1t[:, m2:m2 + 1],
                                              scale=1.0)
                    war_record("py1", [rd])
                    nc.vector.tensor_scalar_add(spe[:, :nh], spe[:, :nh], 1.0)
                    nc.scalar.activation(p1[m2][:, g0_:g0_ + nh], spe[:, :nh],
                                         AF.Ln)
            p2 = [res.tile([128, GTPAD], BF16, tag=f"p2_{m}", name=f"p2_{m}")
                  for m in range(NH)]
            for m2 in range(NH):
                for g0_, nh in ghs:
                    psh = ps_b.tile([128, 512], F32, tag="py1")
                    for kc in range(NH):
                        mm = nc.tensor.matmul(
                            out=psh[:, :nh],
                            lhsT=wp2t[:, kc * FH + m2 * 128:
                                      kc * FH + (m2 + 1) * 128],
                            rhs=p1[kc][:, g0_:g0_ + nh],
                            start=(kc == 0), stop=(kc == NH - 1))
                        if kc == 0:
                            war_deps("py1", 2, mm)
                    spe = work.tile([128, 512], F32, tag="spe", name="spe", bufs=1)
                    rd = nc.scalar.activation(spe[:, :nh], psh[:, :nh], AF.Exp,
                                              bias=bp2t[:, m2:m2 + 1],
                                              scale=1.0)
                    war_record("py1", [rd])
                    nc.vector.tensor_scalar_add(spe[:, :nh], spe[:, :nh], 1.0)
                    nc.scalar.activation(p2[m2][:, g0_:g0_ + nh], spe[:, :nh],
                                         AF.Ln)
            prf = work.tile([2, GTPAD], F32, tag="prf", bufs=1)
            for g0_, nh in ghs:
                psd = ps_c.tile([128, 512], F32, tag="py2", name="ppred")
                for kc in range(NH):
                    mm = nc.tensor.matmul(
                        out=psd[:2, :nh],
                        lhsT=wp3t[:, kc * 2:(kc + 1) * 2],
                        rhs=p2[kc][:, g0_:g0_ + nh],
                        start=(kc == 0), stop=(kc == NH - 1))
                    if kc == 0:
                        war_deps("py2", 2, mm)
                rd = nc.scalar.activation(prf[:, g0_:g0_ + nh], psd[:2, :nh],
                                          AF.Identity, bias=bp3t[:], scale=1.0)
                war_record("py2", [rd])
            nc.sync.dma_start(pred_o[:], prf[:])

    return nc


# --------------------------------------------------------------------------
# host wrapper
# --------------------------------------------------------------------------

def make_in_maps(S, devs, pl):
    L = pl["t15h"].shape[0]
    maps = []
    for d in range(S["ncores"]):
        dv = devs[d]
        # wrapped-16 int16 idx layout, replicated across the 8 Q7 core blocks:
        # slot j of chunk ch at [j % 16, ch * 8 + j // 16]
        g16 = np.zeros((16, S["TOTCH"] * 8), np.int16)
        gi = dv["gidx"]                  # [128, TOTCH] window-local rows
        for ch in range(S["TOTCH"]):
            w16 = gi[:, ch].reshape(8, 16).T    # j at [j%16, j//16]
            g16[:, ch * 8:(ch + 1) * 8] = w16
        gidx16 = np.tile(g16, (8, 1))
        m = dict(
            gidx16=gidx16,
            selTh=np.ascontiguousarray(
                dv["selT"].transpose(1, 0, 2).reshape(128, -1)).astype(BF),
            # (selT is [TOTCH,128,256] -> [128, TOTCH*256])
            cntT=dv["cntT"].astype(BF),
            oneT=dv["oneT"].astype(BF),
            poolh=np.ascontiguousarray(
                dv["poolsel"].transpose(1, 0, 2).reshape(128, -1)).astype(BF),
            t0h=pl["t0h"].astype(BF),
            t15h=pl["t15h"].astype(BF),
            w1h=pl["w1h"].astype(BF),
            w2h=pl["w2h"].astype(BF),
            b1h=pl["b1h"].astype(np.float32),
            b2h=pl["b2h"].astype(np.float32),
            gmh=pl["gmh"].astype(np.float32),
            bth=pl["bth"].astype(np.float32),
            wfh=pl["wfh"].astype(BF),
            bfh=pl["bfh"].astype(np.float32),
            wp1h=pl["wp1h"].astype(BF),
            bp1h=pl["bp1h"].astype(np.float32),
            wp2h=pl["wp2h"].astype(BF),
            bp2h=pl["bp2h"].astype(np.float32),
            wp3h=pl["wp3h"].astype(BF),
            bp3h=pl["bp3h"].astype(np.float32),
            npadv=np.full((128, 1), S["pshard"] - dv["nr"], np.float32),
        )
        maps.append(m)
    return maps


_CACHE = {}


def kernel(**inputs):
    x = np.asarray(inputs["x"])
    edge_index = np.asarray(inputs["edge_index"])
    edge_attr = np.asarray(inputs["edge_attr"])
    batch = np.asarray(inputs["batch"])
    B = 4096
    L, D, FEAT = 5, 300, 512

    S, devs = make_plan(x, edge_index, edge_attr, batch, B)
    pl = layout_params(inputs, L, D, FEAT)
    in_maps = make_in_maps(S, devs, pl)

    key = (S["pshard"], S["TOTCH"], tuple(S["K4"]), S["gtpad"],
           tuple(S["pool_W"]), tuple(S["pool_lo"]))
    if key not in _CACHE:
        nc = bacc.Bacc("TRN2", target_bir_lowering=False, debug=False,
                       num_devices=NCORES)
        build_gnn(nc, S, L, D, FEAT)
        nc.compile()
        _CACHE[key] = nc
    nc = _CACHE[key]

    res = run_bass_kernel_spmd(nc, in_maps, core_ids=list(range(NCORES)))

    hf = np.zeros((B, FEAT), np.float32)
    pred = np.zeros((B, 2), np.float32)
    for d in range(NCORES):
        dv = devs[d]
        g0, gr = dv["g0"], dv["gr"]
        hf[g0:g0 + gr] = res.results[d]["hfT"].T[:gr]
        pred[g0:g0 + gr] = res.results[d]["predT"].T[:gr]
    return hf, pred
